# revision 9
# baseline (speedup 1.0000x reference)
"""ChannelAttention Trainium2 Bass kernel.

Full (unsharded) inputs -> full output. Data-parallel over batch B=8 across
the 8 NeuronCores (one batch element per core, SPMD program, no collectives).

Per-core math (N=4096 tokens, C=512 channels):
    qkv = x @ Wqkv + bqkv ; q,k,v = relu(split(qkv))
    scores = (q^T k) / sqrt(C)           # [C, C] contraction over tokens
    attn = softmax(scores, -1) * adj
    y = v @ attn ; out = y @ Wo + bo

Matmul operands are bf16 (host-cast x/Wqkv/Wo): full PE rate, FWL fast
weight loads, half the DMA/SBUF traffic. Accumulation stays fp32 in PSUM;
softmax is fp32. The q/k biases are added on DVE from broadcast tiles
(materialized once outside the timing loop) instead of rank-1 matmuls,
and the x-transposes are software-pipelined one token-tile ahead of the
projection matmuls.
"""

import sys

sys.path.insert(0, "/opt/trn_rl_repo")

from contextlib import ExitStack

import ml_dtypes
import numpy as np

import concourse.bass as bass
import concourse.mybir as mybir
import concourse.tile as tile
from concourse import bacc
from concourse.bass import ds, ts
from concourse.bass_utils import run_bass_kernel_spmd
from concourse.masks import make_identity

# Problem shape (hardcoded per contract).
B, N, C = 8, 4096, 512
P = 128
CC = C // P            # channel chunks (4)
NT = N // P            # token tiles (32)
TPS = 4                # token tiles per slab
NS = NT // TPS         # slabs (8)
SLAB = TPS * P         # tokens per slab (512)

F32 = mybir.dt.float32
BF16 = mybir.dt.bfloat16
NP_BF16 = ml_dtypes.bfloat16

_CACHE = {}


def build(reps: int = 1, tp_bufs=2, proj_bufs=2, qk_bufs=3,
          xin_bufs=3, xtp_bufs=2, y_bufs=4, pipe_p2=True, hints=True,
          bias_mm=False, x_resident=True, sreset=False):
    nc = bacc.Bacc("TRN2", target_bir_lowering=False, debug=False, num_devices=8)

    x = nc.dram_tensor("x", [N, C], BF16, kind="ExternalInput").ap()
    adj = nc.dram_tensor("adj", [C, C], F32, kind="ExternalInput").ap()
    wqkv = nc.dram_tensor("Wqkv", [C, 3 * C], BF16, kind="ExternalInput").ap()
    bqkv = nc.dram_tensor("bqkv", [3 * C], F32, kind="ExternalInput").ap()
    wo = nc.dram_tensor("Wo", [C, C], BF16, kind="ExternalInput").ap()
    bo = nc.dram_tensor("bo", [C], F32, kind="ExternalInput").ap()
    out = nc.dram_tensor("out", [N, C], F32, kind="ExternalOutput").ap()

    s = 1.0 / float(np.sqrt(C))

    with tile.TileContext(nc) as tc, ExitStack() as ctx:
        const = ctx.enter_context(tc.tile_pool(name="const", bufs=1))

        # ---- constants (weights DMA'd directly as bf16) ----------------
        wqkv_r = const.tile([P, CC, 3 * C], BF16)
        nc.sync.dma_start(wqkv_r[:], wqkv.rearrange("(o p) d -> p o d", p=P))

        wo_r = const.tile([P, CC, C], BF16)
        nc.sync.dma_start(wo_r[:], wo.rearrange("(o p) d -> p o d", p=P))

        ones_r = const.tile([1, P], BF16)
        nc.gpsimd.memset(ones_r[:], 1.0)

        with tc.tile_pool(name="stage", bufs=1) as stage:
            brow_f = stage.tile([1, 3 * C], F32, tag="stage_b")
            nc.sync.dma_start(brow_f[:], bqkv[None, :])
            brow_r = const.tile([1, 3 * C], BF16)
            nc.vector.tensor_copy(brow_r[:], brow_f[:])

            borow_f = stage.tile([1, C], F32, tag="stage_bo")
            nc.sync.dma_start(borow_f[:], bo[None, :])
            borow_r = const.tile([1, C], BF16)
            nc.vector.tensor_copy(borow_r[:], borow_f[:])

        # v-bias, per-partition layout [p, chunk]
        bv = const.tile([P, CC], F32)
        nc.sync.dma_start(bv[:], bqkv[2 * C :].rearrange("(o p) -> p o", p=P))

        ident = const.tile([P, P], BF16)
        make_identity(nc, ident[:])

        adj_sb = const.tile([P, CC, C], F32)
        nc.sync.dma_start(adj_sb[:], adj.rearrange("(o p) d -> p o d", p=P))

        vt_sb = const.tile([P, CC, N], BF16)     # v^T, channel-major
        attn_sb = const.tile([P, CC, C], BF16)   # gated softmax rows

        x_sb = None
        if x_resident:
            # whole x resident in SBUF (32 KiB/partition), loaded once —
            # the rep loop then runs without any input DMA.
            x_sb = const.tile([P, NT, C], BF16)
            nc.sync.dma_start(x_sb[:], x.rearrange("(t p) c -> p t c", p=P))

        # broadcast biases to [P, C] once (outside the rep loop):
        # bq/bk feed DVE adds in pass 1, bo folds into pass-2 evacuation.
        bq_bc = const.tile([P, C], F32)
        bk_bc = const.tile([P, C], F32)
        bo_bc = const.tile([P, C], F32)
        with tc.tile_pool(name="bc_ps", bufs=1, space="PSUM") as bc_ps_pool:
            for bc_sb, brow_ap in (
                (bq_bc, brow_r[:, 0:C]),
                (bk_bc, brow_r[:, C : 2 * C]),
                (bo_bc, borow_r[:]),
            ):
                bc_ps = bc_ps_pool.tile([P, C], F32, tag="bc")
                nc.tensor.matmul(bc_ps[:], ones_r[:], brow_ap, start=True, stop=True)
                nc.vector.tensor_copy(bc_sb[:], bc_ps[:])

        # ---- pass 1: qkv projection + channel scores -------------------
        scores_pool = ctx.enter_context(
            tc.tile_pool(name="scores", bufs=1, space="PSUM")
        )
        scores_ps = [
            scores_pool.tile([P, C], F32, tag=f"scores{o}", name=f"scores{o}")
            for o in range(CC)
        ]

        if reps > 1:
            hint = (
                (mybir.EngineType.PE, mybir.EngineType.DVE,
                 mybir.EngineType.Activation, mybir.EngineType.SP)
                if hints else ()
            )
            ctx.enter_context(
                tc.For_i(0, reps, 1, hint_engines=hint, staggered_reset=sreset)
            )

        with (
            tc.tile_pool(name="tp_ps", bufs=tp_bufs, space="PSUM") as tp_ps,
            tc.tile_pool(name="proj_ps", bufs=proj_bufs, space="PSUM") as proj_ps,
            tc.tile_pool(name="xin", bufs=xin_bufs) as xin,
            tc.tile_pool(name="xtp", bufs=xtp_bufs) as xtp,
            tc.tile_pool(name="qk", bufs=qk_bufs) as qk,
        ):
            xt_slabs = {}

            def get_slab(sl):
                if sl not in xt_slabs:
                    xt_slabs[sl] = xtp.tile(
                        [P, CC, SLAB], BF16, tag="xT", name=f"xt_{sl}"
                    )
                return xt_slabs[sl]

            def emit_transpose(t):
                """Transpose x tile t via PE, evacuate into its slab."""
                sl, tt = divmod(t, TPS)
                if x_resident:
                    x_t = x_sb[:, t, :]
                else:
                    x_t = xin.tile([P, C], BF16, tag="x")[:]
                    nc.sync.dma_start(x_t, x[ts(t, P), :])
                pst = tp_ps.tile([P, C], BF16, tag="tp")
                for o in range(CC):
                    nc.tensor.transpose(pst[:, ts(o, P)], x_t[:, ts(o, P)], ident[:])
                nc.vector.tensor_copy(
                    get_slab(sl)[:, :, ts(tt, P)],
                    pst[:].rearrange("p (o n) -> p o n", o=CC),
                )

            emit_transpose(0)
            for t in range(NT):
                sl, tt = divmod(t, TPS)
                xt_slab = get_slab(sl)
                if t + 1 < NT:
                    emit_transpose(t + 1)

                # q = relu(x @ Wq + bq), k = relu(x @ Wk + bk)
                # (paired per chunk so each stationary x^T tile is
                # loaded once for both projections)
                q_ps = proj_ps.tile([P, C], F32, tag="proj")
                k_ps = proj_ps.tile([P, C], F32, tag="proj")
                for o in range(CC):
                    nc.tensor.matmul(
                        q_ps[:],
                        xt_slab[:, o, ts(tt, P)],
                        wqkv_r[:, o, 0:C],
                        start=(o == 0),
                        stop=(False if bias_mm else o == CC - 1),
                    )
                    nc.tensor.matmul(
                        k_ps[:],
                        xt_slab[:, o, ts(tt, P)],
                        wqkv_r[:, o, C : 2 * C],
                        start=(o == 0),
                        stop=(False if bias_mm else o == CC - 1),
                    )
                if bias_mm:
                    nc.tensor.matmul(
                        q_ps[:], ones_r[:], brow_r[:, 0:C], start=False, stop=True
                    )
                    nc.tensor.matmul(
                        k_ps[:], ones_r[:], brow_r[:, C : 2 * C],
                        start=False, stop=True,
                    )
                    q_sb = qk.tile([P, C], BF16, tag="qs")
                    nc.scalar.activation(
                        q_sb[:], q_ps[:], mybir.ActivationFunctionType.Relu
                    )
                    k_sb = qk.tile([P, C], BF16, tag="ks")
                    nc.vector.tensor_scalar_max(k_sb[:], k_ps[:], 0.0)
                else:
                    q_tmp = qk.tile([P, C], BF16, tag="qt")
                    nc.vector.tensor_tensor(
                        q_tmp[:], q_ps[:], bq_bc[:], mybir.AluOpType.add
                    )
                    q_sb = qk.tile([P, C], BF16, tag="qs")
                    nc.scalar.activation(
                        q_sb[:], q_tmp[:], mybir.ActivationFunctionType.Relu
                    )
                    k_tmp = qk.tile([P, C], BF16, tag="kt")
                    nc.vector.tensor_tensor(
                        k_tmp[:], k_ps[:], bk_bc[:], mybir.AluOpType.add
                    )
                    k_sb = qk.tile([P, C], BF16, tag="ks")
                    nc.vector.tensor_scalar_max(k_sb[:], k_tmp[:], 0.0)

                # scores[o] += q[:, o-chunk]^T @ k
                for o in range(CC):
                    nc.tensor.matmul(
                        scores_ps[o][:],
                        q_sb[:, ts(o, P)],
                        k_sb[:],
                        start=(t == 0),
                        stop=(t == NT - 1),
                    )

                if tt == TPS - 1:
                    # vT[d, n] = relu(Wv^T x^T + bv)  (channel-major)
                    for d in range(CC):
                        v_ps = proj_ps.tile([P, C], F32, tag="proj")
                        for o in range(CC):
                            nc.tensor.matmul(
                                v_ps[:, :SLAB],
                                wqkv_r[:, o, ds(2 * C + d * P, P)],
                                xt_slab[:, o, :],
                                start=(o == 0),
                                stop=(o == CC - 1),
                            )
                        nc.scalar.activation(
                            vt_sb[:, d, ts(sl, SLAB)],
                            v_ps[:, :SLAB],
                            mybir.ActivationFunctionType.Relu,
                            bias=bv[:, d : d + 1],
                        )
                    del xt_slabs[sl]

            # ---- softmax + adjacency gate ------------------------------
            with tc.tile_pool(name="smx", bufs=8) as smx:
                for o in range(CC):
                    smax = smx.tile([P, 1], F32, tag="smax")
                    nc.vector.reduce_max(
                        smax[:], scores_ps[o][:], axis=mybir.AxisListType.X
                    )
                    nbias = smx.tile([P, 1], F32, tag="nbias")
                    nc.vector.tensor_scalar_mul(nbias[:], smax[:], -s)
                    ssum = smx.tile([P, 1], F32, tag="ssum")
                    attn_e = smx.tile([P, C], F32, tag="attn_e")
                    nc.scalar.activation(
                        attn_e[:],
                        scores_ps[o][:],
                        mybir.ActivationFunctionType.Exp,
                        bias=nbias[:],
                        scale=s,
                        accum_out=ssum[:],
                    )
                    rsum = smx.tile([P, 1], F32, tag="rsum")
                    nc.vector.reciprocal(rsum[:], ssum[:])
                    attn_r = smx.tile([P, C], F32, tag="attn_r")
                    nc.vector.tensor_scalar_mul(attn_r[:], attn_e[:], rsum[:])
                    nc.vector.tensor_mul(
                        attn_sb[:, o, :], attn_r[:], adj_sb[:, o, :]
                    )

        # ---- pass 2: y = v @ attn ; out = y @ Wo + bo ------------------
        with (
            tc.tile_pool(name="y_ps", bufs=y_bufs, space="PSUM") as y_ps_pool,
            tc.tile_pool(name="yt", bufs=2) as ytp,
            tc.tile_pool(name="outp", bufs=3) as outp,
        ):
            def emit_yt(sl):
                yt_slab = ytp.tile([P, CC, SLAB], BF16, tag="yT", name=f"yt_{sl}")
                for d in range(CC):
                    y_ps = y_ps_pool.tile([P, C], F32, tag="y", name=f"y_{sl}_{d}")
                    for o in range(CC):
                        nc.tensor.matmul(
                            y_ps[:, :SLAB],
                            attn_sb[:, o, ts(d, P)],
                            vt_sb[:, o, ts(sl, SLAB)],
                            start=(o == 0),
                            stop=(o == CC - 1),
                        )
                    nc.scalar.copy(yt_slab[:, d, :], y_ps[:, :SLAB])
                return yt_slab

            def emit_out(sl, yt_slab):
                for tt in range(TPS):
                    t = sl * TPS + tt
                    o_ps = y_ps_pool.tile([P, C], F32, tag="y", name=f"o_{sl}_{tt}")
                    for d in range(CC):
                        nc.tensor.matmul(
                            o_ps[:],
                            yt_slab[:, d, ts(tt, P)],
                            wo_r[:, d, :],
                            start=(d == 0),
                            stop=(d == CC - 1),
                        )
                    out_sb = outp.tile([P, C], F32, tag="out", name=f"os_{sl}_{tt}")
                    nc.vector.tensor_tensor(
                        out_sb[:], o_ps[:], bo_bc[:], mybir.AluOpType.add
                    )
                    nc.sync.dma_start(out[ts(t, P), :], out_sb[:])

            if pipe_p2:
                prev = None
                for sl in range(NS):
                    yt_slab = emit_yt(sl)
                    if prev is not None:
                        emit_out(sl - 1, prev)
                    prev = yt_slab
                emit_out(NS - 1, prev)
            else:
                for sl in range(NS):
                    emit_out(sl, emit_yt(sl))

    nc.compile()
    return nc


def _get_nc(reps: int = 1, **kw):
    key = ("nc", reps, tuple(sorted(kw.items())))
    if key not in _CACHE:
        _CACHE[key] = build(reps, **kw)
    return _CACHE[key]


def _run(inputs, trace=False, reps: int = 1, **kw):
    nc = _get_nc(reps, **kw)
    x = np.ascontiguousarray(np.asarray(inputs["x"], dtype=np.float32)).astype(NP_BF16)
    adj = np.ascontiguousarray(np.asarray(inputs["adj"], dtype=np.float32))
    wqkv = np.ascontiguousarray(np.asarray(inputs["Wqkv"], dtype=np.float32)).astype(NP_BF16)
    bqkv = np.ascontiguousarray(np.asarray(inputs["bqkv"], dtype=np.float32))
    wo = np.ascontiguousarray(np.asarray(inputs["Wo"], dtype=np.float32)).astype(NP_BF16)
    bo = np.ascontiguousarray(np.asarray(inputs["bo"], dtype=np.float32))

    in_maps = [
        {
            "x": x[b],
            "adj": adj[b],
            "Wqkv": wqkv,
            "bqkv": bqkv,
            "Wo": wo,
            "bo": bo,
        }
        for b in range(B)
    ]
    res = run_bass_kernel_spmd(
        nc, in_maps, core_ids=list(range(B)), trace=trace
    )
    outp = np.stack([res.results[b]["out"] for b in range(B)], axis=0)
    return outp.astype(np.float32), res


def kernel(**inputs) -> np.ndarray:
    out, _ = _run(inputs, trace=False)
    return out


# revision 39
# speedup vs baseline: 1.5923x; 1.5923x over previous
"""ChannelAttention Trainium2 Bass kernel.

Full (unsharded) inputs -> full output. Data-parallel over batch B=8 across
the 8 NeuronCores (one batch element per core, SPMD program, no collectives).

Per-core math (N=4096 tokens, C=512 channels):
    qkv = x @ Wqkv + bqkv ; q,k,v = relu(split(qkv))
    scores = (q^T k) / sqrt(C)           # [C, C] contraction over tokens
    attn = softmax(scores, -1) * adj
    y = v @ attn ; out = y @ Wo + bo

Matmul operands are bf16 (host-cast x/Wqkv/Wo): full PE rate, FWL fast
weight loads, half the DMA/SBUF traffic. Accumulation stays fp32 in PSUM;
softmax is fp32. The q/k biases are added on DVE from broadcast tiles
(materialized once outside the timing loop) instead of rank-1 matmuls,
and the x-transposes are software-pipelined one token-tile ahead of the
projection matmuls.
"""

import sys

sys.path.insert(0, "/opt/trn_rl_repo")

from contextlib import ExitStack

import ml_dtypes
import numpy as np

import concourse.bass as bass
import concourse.mybir as mybir
import concourse.tile as tile
from concourse import bacc
from concourse.bass import ds, ts
from concourse.bass_utils import run_bass_kernel_spmd
from concourse.masks import make_identity

# Problem shape (hardcoded per contract).
B, N, C = 8, 4096, 512
P = 128
CC = C // P            # channel chunks (4)
NT = N // P            # token tiles (32)
TPS = 4                # token tiles per slab
NS = NT // TPS         # slabs (8)
SLAB = TPS * P         # tokens per slab (512)

F32 = mybir.dt.float32
BF16 = mybir.dt.bfloat16
FP8 = mybir.dt.float8e4
NP_BF16 = ml_dtypes.bfloat16
NP_FP8 = ml_dtypes.float8_e4m3fn
FP8_WSCALE = 16.0
ATTN_SCALE = 256.0

_CACHE = {}

# Winning configuration (A/B-benched on hardware): fp8 DoubleRow for the
# projection, scores, and y matmuls; rank-1 matmul bias adds for q/k; PSUM
# split 4 scores + 1 transpose + 3 projection banks; x streamed per tile.
DEFAULTS = dict(
    full_fp8=True, bias_mm=True, tp_bufs=1, proj_bufs=3, x_resident=False
)


def build(reps: int = 1, tp_bufs=2, proj_bufs=2, qk_bufs=3,
          xin_bufs=3, xtp_bufs=2, y_bufs=4, pipe_p2=True, hints=True,
          bias_mm=False, x_resident=True, sreset=False, proj_fp8=False,
          full_fp8=False):
    proj_fp8 = proj_fp8 or full_fp8
    nc = bacc.Bacc("TRN2", target_bir_lowering=False, debug=False, num_devices=8)

    # x and the PE transposes stay bf16 (fp8 transpose trips a walrus
    # verifier rule); the PSUM->SBUF evacuation copy casts x^T to fp8.
    XDT = FP8 if proj_fp8 else BF16
    x = nc.dram_tensor("x", [N, C], BF16, kind="ExternalInput").ap()
    adj = nc.dram_tensor("adj", [C, C], F32, kind="ExternalInput").ap()
    wqkv = nc.dram_tensor("Wqkv", [C, 3 * C], XDT, kind="ExternalInput").ap()
    bqkv = nc.dram_tensor("bqkv", [3 * C], F32, kind="ExternalInput").ap()
    wo = nc.dram_tensor("Wo", [C, C], BF16, kind="ExternalInput").ap()
    bo = nc.dram_tensor("bo", [C], F32, kind="ExternalInput").ap()
    out = nc.dram_tensor("out", [N, C], F32, kind="ExternalOutput").ap()

    s = 1.0 / float(np.sqrt(C))

    with tile.TileContext(nc) as tc, ExitStack() as ctx:
        const = ctx.enter_context(tc.tile_pool(name="const", bufs=1))

        # ---- constants (weights DMA'd directly as bf16) ----------------
        wqkv_r = const.tile([P, CC, 3 * C], XDT)
        nc.sync.dma_start(wqkv_r[:], wqkv.rearrange("(o p) d -> p o d", p=P))

        wo_r = const.tile([P, CC, C], BF16)
        nc.sync.dma_start(wo_r[:], wo.rearrange("(o p) d -> p o d", p=P))

        ones_r = const.tile([1, P], BF16)
        nc.gpsimd.memset(ones_r[:], 1.0)

        with tc.tile_pool(name="stage", bufs=1) as stage:
            brow_f = stage.tile([1, 3 * C], F32, tag="stage_b")
            nc.sync.dma_start(brow_f[:], bqkv[None, :])
            brow_r = const.tile([1, 3 * C], BF16)
            if proj_fp8:
                # q/k bias rows pre-scaled to match the x16 fp8 weights
                nc.vector.tensor_scalar_mul(brow_r[:], brow_f[:], FP8_WSCALE)
            else:
                nc.vector.tensor_copy(brow_r[:], brow_f[:])

            borow_f = stage.tile([1, C], F32, tag="stage_bo")
            nc.sync.dma_start(borow_f[:], bo[None, :])
            borow_r = const.tile([1, C], BF16)
            nc.vector.tensor_copy(borow_r[:], borow_f[:])

        # v-bias, per-partition layout [p, chunk]
        bv = const.tile([P, CC], F32)
        nc.sync.dma_start(bv[:], bqkv[2 * C :].rearrange("(o p) -> p o", p=P))

        ident = const.tile([P, P], BF16)
        make_identity(nc, ident[:])

        adj_sb = const.tile([P, CC, C], F32)
        nc.sync.dma_start(adj_sb[:], adj.rearrange("(o p) d -> p o d", p=P))

        # In full_fp8 mode attn carries a x256 scale (host-folded into adj;
        # undone by Wo/256) and v/q/k/attn run the y/scores matmuls in fp8.
        SDT = FP8 if full_fp8 else BF16
        vt_sb = const.tile([P, CC, N], SDT)      # v^T, channel-major
        attn_sb = const.tile([P, CC, C], SDT)    # gated softmax rows

        x_sb = None
        if x_resident:
            # whole x resident in SBUF (32 KiB/partition), loaded once —
            # the rep loop then runs without any input DMA.
            x_sb = const.tile([P, NT, C], BF16)
            nc.sync.dma_start(x_sb[:], x.rearrange("(t p) c -> p t c", p=P))

        # broadcast biases to [P, C] once (outside the rep loop):
        # bq/bk feed DVE adds in pass 1, bo folds into pass-2 evacuation.
        bq_bc = const.tile([P, C], F32)
        bk_bc = const.tile([P, C], F32)
        bo_bc = const.tile([P, C], F32)
        with tc.tile_pool(name="bc_ps", bufs=1, space="PSUM") as bc_ps_pool:
            for bc_sb, brow_ap in (
                (bq_bc, brow_r[:, 0:C]),
                (bk_bc, brow_r[:, C : 2 * C]),
                (bo_bc, borow_r[:]),
            ):
                bc_ps = bc_ps_pool.tile([P, C], F32, tag="bc")
                nc.tensor.matmul(bc_ps[:], ones_r[:], brow_ap, start=True, stop=True)
                nc.vector.tensor_copy(bc_sb[:], bc_ps[:])

        # ---- pass 1: qkv projection + channel scores -------------------
        scores_pool = ctx.enter_context(
            tc.tile_pool(name="scores", bufs=1, space="PSUM")
        )
        scores_ps = [
            scores_pool.tile([P, C], F32, tag=f"scores{o}", name=f"scores{o}")
            for o in range(CC)
        ]

        if reps > 1:
            hint = (
                (mybir.EngineType.PE, mybir.EngineType.DVE,
                 mybir.EngineType.Activation, mybir.EngineType.SP)
                if hints else ()
            )
            ctx.enter_context(
                tc.For_i(0, reps, 1, hint_engines=hint, staggered_reset=sreset)
            )

        with (
            tc.tile_pool(name="tp_ps", bufs=tp_bufs, space="PSUM") as tp_ps,
            tc.tile_pool(name="proj_ps", bufs=proj_bufs, space="PSUM") as proj_ps,
            tc.tile_pool(name="xin", bufs=xin_bufs) as xin,
            tc.tile_pool(name="xtp", bufs=xtp_bufs) as xtp,
            tc.tile_pool(name="qk", bufs=qk_bufs) as qk,
        ):
            xt_slabs = {}

            def get_slab(sl):
                if sl not in xt_slabs:
                    xt_slabs[sl] = xtp.tile(
                        [P, CC, SLAB], XDT, tag="xT", name=f"xt_{sl}"
                    )
                return xt_slabs[sl]

            def emit_transpose(t):
                """Transpose x tile t via PE, evacuate into its slab."""
                sl, tt = divmod(t, TPS)
                if x_resident:
                    x_t = x_sb[:, t, :]
                else:
                    x_tile = xin.tile([P, C], BF16, tag="x", name=f"x_{t}")
                    x_t = x_tile[:]
                    nc.sync.dma_start(x_t, x[ts(t, P), :])
                pst = tp_ps.tile([P, C], BF16, tag="tp")
                for o in range(CC):
                    nc.tensor.transpose(pst[:, ts(o, P)], x_t[:, ts(o, P)], ident[:])
                nc.vector.tensor_copy(
                    get_slab(sl)[:, :, ts(tt, P)],
                    pst[:].rearrange("p (o n) -> p o n", o=CC),
                )

            emit_transpose(0)
            for t in range(NT):
                sl, tt = divmod(t, TPS)
                xt_slab = get_slab(sl)
                if t + 1 < NT:
                    emit_transpose(t + 1)

                # q = relu(x @ Wq + bq), k = relu(x @ Wk + bk)
                # (paired per chunk so each stationary x^T tile is
                # loaded once for both projections)
                q_ps = proj_ps.tile([P, C], F32, tag="proj")
                k_ps = proj_ps.tile([P, C], F32, tag="proj")
                if proj_fp8:
                    # DoubleRow: contract two 128-channel chunks per matmul
                    for o2 in range(CC // 2):
                        o = 2 * o2
                        last = False if bias_mm else (o2 == CC // 2 - 1)
                        nc.tensor.matmul(
                            q_ps[:],
                            xt_slab[:, o : o + 2, ts(tt, P)],
                            wqkv_r[:, o : o + 2, 0:C],
                            start=(o2 == 0),
                            stop=last,
                            perf_mode=mybir.MatmulPerfMode.DoubleRow,
                        )
                        nc.tensor.matmul(
                            k_ps[:],
                            xt_slab[:, o : o + 2, ts(tt, P)],
                            wqkv_r[:, o : o + 2, C : 2 * C],
                            start=(o2 == 0),
                            stop=last,
                            perf_mode=mybir.MatmulPerfMode.DoubleRow,
                        )
                else:
                    for o in range(CC):
                        nc.tensor.matmul(
                            q_ps[:],
                            xt_slab[:, o, ts(tt, P)],
                            wqkv_r[:, o, 0:C],
                            start=(o == 0),
                            stop=(False if bias_mm else o == CC - 1),
                        )
                        nc.tensor.matmul(
                            k_ps[:],
                            xt_slab[:, o, ts(tt, P)],
                            wqkv_r[:, o, C : 2 * C],
                            start=(o == 0),
                            stop=(False if bias_mm else o == CC - 1),
                        )
                # evacuation destinations: flat [P, C] tiles normally, or
                # slot t%2 of a token-tile-pair [P, 2, C] fp8 tile so the
                # scores matmuls can contract both tiles with DoubleRow
                if full_fp8:
                    if t % 2 == 0:
                        pair_tiles = (
                            qk.tile([P, 2, C], FP8, tag="qpair", name=f"qp_{t}"),
                            qk.tile([P, 2, C], FP8, tag="kpair", name=f"kp_{t}"),
                        )
                    q_pair, k_pair = pair_tiles
                    q_dst = q_pair[:, t % 2, :]
                    k_dst = k_pair[:, t % 2, :]
                else:
                    q_sb = qk.tile([P, C], BF16, tag="qs")
                    k_sb = qk.tile([P, C], BF16, tag="ks")
                    q_dst = q_sb[:]
                    k_dst = k_sb[:]

                dsc = 1.0 / FP8_WSCALE
                if bias_mm:
                    nc.tensor.matmul(
                        q_ps[:], ones_r[:], brow_r[:, 0:C], start=False, stop=True
                    )
                    nc.tensor.matmul(
                        k_ps[:], ones_r[:], brow_r[:, C : 2 * C],
                        start=False, stop=True,
                    )
                    if proj_fp8:
                        nc.scalar.activation(
                            q_dst, q_ps[:], mybir.ActivationFunctionType.Relu,
                            scale=dsc,
                        )
                        nc.vector.tensor_scalar(
                            k_dst, k_ps[:], 0.0, dsc,
                            mybir.AluOpType.max, mybir.AluOpType.mult,
                        )
                    else:
                        nc.scalar.activation(
                            q_dst, q_ps[:], mybir.ActivationFunctionType.Relu
                        )
                        nc.vector.tensor_scalar_max(k_dst, k_ps[:], 0.0)
                else:
                    # (for proj_fp8 the PSUM holds 16x the true pre-bias
                    # values and bq/bk_bc were pre-scaled to match; relu's
                    # positive homogeneity lets us descale after the max)
                    q_tmp = qk.tile([P, C], BF16, tag="qt")
                    nc.vector.tensor_tensor(
                        q_tmp[:], q_ps[:], bq_bc[:], mybir.AluOpType.add
                    )
                    if proj_fp8:
                        nc.scalar.activation(
                            q_dst, q_tmp[:],
                            mybir.ActivationFunctionType.Relu, scale=dsc,
                        )
                    else:
                        nc.scalar.activation(
                            q_dst, q_tmp[:], mybir.ActivationFunctionType.Relu
                        )
                    k_tmp = qk.tile([P, C], BF16, tag="kt")
                    nc.vector.tensor_tensor(
                        k_tmp[:], k_ps[:], bk_bc[:], mybir.AluOpType.add
                    )
                    if proj_fp8:
                        nc.vector.tensor_scalar(
                            k_dst, k_tmp[:], 0.0, dsc,
                            mybir.AluOpType.max, mybir.AluOpType.mult,
                        )
                    else:
                        nc.vector.tensor_scalar_max(k_dst, k_tmp[:], 0.0)

                # scores[o] += q[:, o-chunk]^T @ k
                if full_fp8:
                    if t % 2 == 1:
                        for o in range(CC):
                            nc.tensor.matmul(
                                scores_ps[o][:],
                                q_pair[:, :, ts(o, P)],
                                k_pair[:, :, :],
                                start=(t == 1),
                                stop=(t == NT - 1),
                                perf_mode=mybir.MatmulPerfMode.DoubleRow,
                            )
                else:
                    for o in range(CC):
                        nc.tensor.matmul(
                            scores_ps[o][:],
                            q_sb[:, ts(o, P)],
                            k_sb[:],
                            start=(t == 0),
                            stop=(t == NT - 1),
                        )

                if tt == TPS - 1:
                    # vT[d, n] = relu(Wv^T x^T + bv)  (channel-major)
                    for d in range(CC):
                        v_ps = proj_ps.tile([P, C], F32, tag="proj")
                        if proj_fp8:
                            for o2 in range(CC // 2):
                                o = 2 * o2
                                nc.tensor.matmul(
                                    v_ps[:, :SLAB],
                                    wqkv_r[:, o : o + 2, ds(2 * C + d * P, P)],
                                    xt_slab[:, o : o + 2, :],
                                    start=(o2 == 0),
                                    stop=(o2 == CC // 2 - 1),
                                    perf_mode=mybir.MatmulPerfMode.DoubleRow,
                                )
                            nc.scalar.activation(
                                vt_sb[:, d, ts(sl, SLAB)],
                                v_ps[:, :SLAB],
                                mybir.ActivationFunctionType.Relu,
                                bias=bv[:, d : d + 1],
                                scale=1.0 / FP8_WSCALE,
                            )
                        else:
                            for o in range(CC):
                                nc.tensor.matmul(
                                    v_ps[:, :SLAB],
                                    wqkv_r[:, o, ds(2 * C + d * P, P)],
                                    xt_slab[:, o, :],
                                    start=(o == 0),
                                    stop=(o == CC - 1),
                                )
                            nc.scalar.activation(
                                vt_sb[:, d, ts(sl, SLAB)],
                                v_ps[:, :SLAB],
                                mybir.ActivationFunctionType.Relu,
                                bias=bv[:, d : d + 1],
                            )
                    del xt_slabs[sl]

            # ---- softmax + adjacency gate ------------------------------
            with tc.tile_pool(name="smx", bufs=8) as smx:
                for o in range(CC):
                    smax = smx.tile([P, 1], F32, tag="smax")
                    nc.vector.reduce_max(
                        smax[:], scores_ps[o][:], axis=mybir.AxisListType.X
                    )
                    nbias = smx.tile([P, 1], F32, tag="nbias")
                    nc.vector.tensor_scalar_mul(nbias[:], smax[:], -s)
                    ssum = smx.tile([P, 1], F32, tag="ssum")
                    attn_e = smx.tile([P, C], F32, tag="attn_e")
                    nc.scalar.activation(
                        attn_e[:],
                        scores_ps[o][:],
                        mybir.ActivationFunctionType.Exp,
                        bias=nbias[:],
                        scale=s,
                        accum_out=ssum[:],
                    )
                    rsum = smx.tile([P, 1], F32, tag="rsum")
                    nc.vector.reciprocal(rsum[:], ssum[:])
                    attn_r = smx.tile([P, C], F32, tag="attn_r")
                    nc.vector.tensor_scalar_mul(attn_r[:], attn_e[:], rsum[:])
                    nc.vector.tensor_mul(
                        attn_sb[:, o, :], attn_r[:], adj_sb[:, o, :]
                    )

        # ---- pass 2: y = v @ attn ; out = y @ Wo + bo ------------------
        with (
            tc.tile_pool(name="y_ps", bufs=y_bufs, space="PSUM") as y_ps_pool,
            tc.tile_pool(name="yt", bufs=2) as ytp,
            tc.tile_pool(name="outp", bufs=3) as outp,
        ):
            def emit_yt(sl):
                yt_slab = ytp.tile([P, CC, SLAB], BF16, tag="yT", name=f"yt_{sl}")
                for d in range(CC):
                    y_ps = y_ps_pool.tile([P, C], F32, tag="y", name=f"y_{sl}_{d}")
                    if full_fp8:
                        for o2 in range(CC // 2):
                            o = 2 * o2
                            nc.tensor.matmul(
                                y_ps[:, :SLAB],
                                attn_sb[:, o : o + 2, ts(d, P)],
                                vt_sb[:, o : o + 2, ts(sl, SLAB)],
                                start=(o2 == 0),
                                stop=(o2 == CC // 2 - 1),
                                perf_mode=mybir.MatmulPerfMode.DoubleRow,
                            )
                    else:
                        for o in range(CC):
                            nc.tensor.matmul(
                                y_ps[:, :SLAB],
                                attn_sb[:, o, ts(d, P)],
                                vt_sb[:, o, ts(sl, SLAB)],
                                start=(o == 0),
                                stop=(o == CC - 1),
                            )
                    nc.scalar.copy(yt_slab[:, d, :], y_ps[:, :SLAB])
                return yt_slab

            def emit_out(sl, yt_slab):
                for tt in range(TPS):
                    t = sl * TPS + tt
                    o_ps = y_ps_pool.tile([P, C], F32, tag="y", name=f"o_{sl}_{tt}")
                    for d in range(CC):
                        nc.tensor.matmul(
                            o_ps[:],
                            yt_slab[:, d, ts(tt, P)],
                            wo_r[:, d, :],
                            start=(d == 0),
                            stop=(d == CC - 1),
                        )
                    out_sb = outp.tile([P, C], F32, tag="out", name=f"os_{sl}_{tt}")
                    nc.vector.tensor_tensor(
                        out_sb[:], o_ps[:], bo_bc[:], mybir.AluOpType.add
                    )
                    nc.sync.dma_start(out[ts(t, P), :], out_sb[:])

            if pipe_p2:
                prev = None
                for sl in range(NS):
                    yt_slab = emit_yt(sl)
                    if prev is not None:
                        emit_out(sl - 1, prev)
                    prev = yt_slab
                emit_out(NS - 1, prev)
            else:
                for sl in range(NS):
                    emit_out(sl, emit_yt(sl))

    nc.compile()
    return nc


def _get_nc(reps: int = 1, **kw):
    eff = {**DEFAULTS, **kw}
    key = ("nc", reps, tuple(sorted(eff.items())))
    if key not in _CACHE:
        _CACHE[key] = build(reps, **eff)
    return _CACHE[key]


def _run(inputs, trace=False, reps: int = 1, **kw):
    eff = {**DEFAULTS, **kw}
    nc = _get_nc(reps, **kw)
    xf = np.ascontiguousarray(np.asarray(inputs["x"], dtype=np.float32))
    wqkvf = np.ascontiguousarray(np.asarray(inputs["Wqkv"], dtype=np.float32))
    x = xf.astype(NP_BF16)
    if eff.get("proj_fp8") or eff.get("full_fp8"):
        wqkv = (wqkvf * FP8_WSCALE).astype(NP_FP8)
    else:
        wqkv = wqkvf.astype(NP_BF16)
    adj = np.ascontiguousarray(np.asarray(inputs["adj"], dtype=np.float32))
    bqkv = np.ascontiguousarray(np.asarray(inputs["bqkv"], dtype=np.float32))
    wof = np.ascontiguousarray(np.asarray(inputs["Wo"], dtype=np.float32))
    if eff.get("full_fp8"):
        # attn carries x256 (folded into adj); undo via Wo/256
        adj = adj * ATTN_SCALE
        wo = (wof / ATTN_SCALE).astype(NP_BF16)
    else:
        wo = wof.astype(NP_BF16)
    bo = np.ascontiguousarray(np.asarray(inputs["bo"], dtype=np.float32))

    in_maps = [
        {
            "x": x[b],
            "adj": adj[b],
            "Wqkv": wqkv,
            "bqkv": bqkv,
            "Wo": wo,
            "bo": bo,
        }
        for b in range(B)
    ]
    res = run_bass_kernel_spmd(
        nc, in_maps, core_ids=list(range(B)), trace=trace
    )
    outp = np.stack([res.results[b]["out"] for b in range(B)], axis=0)
    return outp.astype(np.float32), res


def kernel(**inputs) -> np.ndarray:
    out, _ = _run(inputs, trace=False)
    return out


# revision 43
# speedup vs baseline: 1.5970x; 1.0030x over previous
"""ChannelAttention Trainium2 Bass kernel.

Full (unsharded) inputs -> full output. Data-parallel over batch B=8 across
the 8 NeuronCores (one batch element per core, SPMD program, no collectives).

Per-core math (N=4096 tokens, C=512 channels):
    qkv = x @ Wqkv + bqkv ; q,k,v = relu(split(qkv))
    scores = (q^T k) / sqrt(C)           # [C, C] contraction over tokens
    attn = softmax(scores, -1) * adj
    y = v @ attn ; out = y @ Wo + bo

Matmul operands are bf16 (host-cast x/Wqkv/Wo): full PE rate, FWL fast
weight loads, half the DMA/SBUF traffic. Accumulation stays fp32 in PSUM;
softmax is fp32. The q/k biases are added on DVE from broadcast tiles
(materialized once outside the timing loop) instead of rank-1 matmuls,
and the x-transposes are software-pipelined one token-tile ahead of the
projection matmuls.
"""

import sys

sys.path.insert(0, "/opt/trn_rl_repo")

from contextlib import ExitStack

import ml_dtypes
import numpy as np

import concourse.bass as bass
import concourse.mybir as mybir
import concourse.tile as tile
from concourse import bacc
from concourse.bass import ds, ts
from concourse.bass_utils import run_bass_kernel_spmd
from concourse.masks import make_identity

# Problem shape (hardcoded per contract).
B, N, C = 8, 4096, 512
P = 128
CC = C // P            # channel chunks (4)
NT = N // P            # token tiles (32)
TPS = 4                # token tiles per slab
NS = NT // TPS         # slabs (8)
SLAB = TPS * P         # tokens per slab (512)

F32 = mybir.dt.float32
BF16 = mybir.dt.bfloat16
FP8 = mybir.dt.float8e4
NP_BF16 = ml_dtypes.bfloat16
NP_FP8 = ml_dtypes.float8_e4m3fn
FP8_WSCALE = 16.0
ATTN_SCALE = 256.0

_CACHE = {}

# Winning configuration (A/B-benched on hardware): fp8 DoubleRow for the
# projection, scores, and y matmuls; rank-1 matmul bias adds for q/k; PSUM
# split 4 scores + 1 transpose + 3 projection banks; x streamed per tile.
DEFAULTS = dict(
    full_fp8=True, bias_mm=True, tp_bufs=1, proj_bufs=3, x_resident=False
)


def build(reps: int = 1, tp_bufs=2, proj_bufs=2, qk_bufs=3,
          xin_bufs=3, xtp_bufs=2, y_bufs=4, pipe_p2=True, hints=True,
          bias_mm=False, x_resident=True, sreset=False, proj_fp8=False,
          full_fp8=False, ilv=False):
    proj_fp8 = proj_fp8 or full_fp8
    nc = bacc.Bacc("TRN2", target_bir_lowering=False, debug=False, num_devices=8)

    # x and the PE transposes stay bf16 (fp8 transpose trips a walrus
    # verifier rule); the PSUM->SBUF evacuation copy casts x^T to fp8.
    XDT = FP8 if proj_fp8 else BF16
    x = nc.dram_tensor("x", [N, C], BF16, kind="ExternalInput").ap()
    adj = nc.dram_tensor("adj", [C, C], F32, kind="ExternalInput").ap()
    wqkv = nc.dram_tensor("Wqkv", [C, 3 * C], XDT, kind="ExternalInput").ap()
    bqkv = nc.dram_tensor("bqkv", [3 * C], F32, kind="ExternalInput").ap()
    wo = nc.dram_tensor("Wo", [C, C], BF16, kind="ExternalInput").ap()
    bo = nc.dram_tensor("bo", [C], F32, kind="ExternalInput").ap()
    out = nc.dram_tensor("out", [N, C], F32, kind="ExternalOutput").ap()

    s = 1.0 / float(np.sqrt(C))

    with tile.TileContext(nc) as tc, ExitStack() as ctx:
        const = ctx.enter_context(tc.tile_pool(name="const", bufs=1))

        # ---- constants (weights DMA'd directly as bf16) ----------------
        wqkv_r = const.tile([P, CC, 3 * C], XDT)
        nc.sync.dma_start(wqkv_r[:], wqkv.rearrange("(o p) d -> p o d", p=P))

        wo_r = const.tile([P, CC, C], BF16)
        nc.sync.dma_start(wo_r[:], wo.rearrange("(o p) d -> p o d", p=P))

        ones_r = const.tile([1, P], BF16)
        nc.gpsimd.memset(ones_r[:], 1.0)

        with tc.tile_pool(name="stage", bufs=1) as stage:
            brow_f = stage.tile([1, 3 * C], F32, tag="stage_b")
            nc.sync.dma_start(brow_f[:], bqkv[None, :])
            brow_r = const.tile([1, 3 * C], BF16)
            if proj_fp8:
                # q/k bias rows pre-scaled to match the x16 fp8 weights
                nc.vector.tensor_scalar_mul(brow_r[:], brow_f[:], FP8_WSCALE)
            else:
                nc.vector.tensor_copy(brow_r[:], brow_f[:])

            borow_f = stage.tile([1, C], F32, tag="stage_bo")
            nc.sync.dma_start(borow_f[:], bo[None, :])
            borow_r = const.tile([1, C], BF16)
            nc.vector.tensor_copy(borow_r[:], borow_f[:])

        # v-bias, per-partition layout [p, chunk]
        bv = const.tile([P, CC], F32)
        nc.sync.dma_start(bv[:], bqkv[2 * C :].rearrange("(o p) -> p o", p=P))

        ident = const.tile([P, P], BF16)
        make_identity(nc, ident[:])

        adj_sb = const.tile([P, CC, C], F32)
        nc.sync.dma_start(adj_sb[:], adj.rearrange("(o p) d -> p o d", p=P))

        # In full_fp8 mode attn carries a x256 scale (host-folded into adj;
        # undone by Wo/256) and v/q/k/attn run the y/scores matmuls in fp8.
        SDT = FP8 if full_fp8 else BF16
        vt_sb = const.tile([P, CC, N], SDT)      # v^T, channel-major
        attn_sb = const.tile([P, CC, C], SDT)    # gated softmax rows

        x_sb = None
        if x_resident:
            # whole x resident in SBUF (32 KiB/partition), loaded once —
            # the rep loop then runs without any input DMA.
            x_sb = const.tile([P, NT, C], BF16)
            nc.sync.dma_start(x_sb[:], x.rearrange("(t p) c -> p t c", p=P))

        # broadcast biases to [P, C] once (outside the rep loop):
        # bq/bk feed DVE adds in pass 1, bo folds into pass-2 evacuation.
        bq_bc = const.tile([P, C], F32)
        bk_bc = const.tile([P, C], F32)
        bo_bc = const.tile([P, C], F32)
        with tc.tile_pool(name="bc_ps", bufs=1, space="PSUM") as bc_ps_pool:
            for bc_sb, brow_ap in (
                (bq_bc, brow_r[:, 0:C]),
                (bk_bc, brow_r[:, C : 2 * C]),
                (bo_bc, borow_r[:]),
            ):
                bc_ps = bc_ps_pool.tile([P, C], F32, tag="bc")
                nc.tensor.matmul(bc_ps[:], ones_r[:], brow_ap, start=True, stop=True)
                nc.vector.tensor_copy(bc_sb[:], bc_ps[:])

        # ---- pass 1: qkv projection + channel scores -------------------
        scores_pool = ctx.enter_context(
            tc.tile_pool(name="scores", bufs=1, space="PSUM")
        )
        scores_ps = [
            scores_pool.tile([P, C], F32, tag=f"scores{o}", name=f"scores{o}")
            for o in range(CC)
        ]

        if reps > 1:
            hint = (
                (mybir.EngineType.PE, mybir.EngineType.DVE,
                 mybir.EngineType.Activation, mybir.EngineType.SP)
                if hints else ()
            )
            ctx.enter_context(
                tc.For_i(0, reps, 1, hint_engines=hint, staggered_reset=sreset)
            )

        with (
            tc.tile_pool(name="tp_ps", bufs=tp_bufs, space="PSUM") as tp_ps,
            tc.tile_pool(name="proj_ps", bufs=proj_bufs, space="PSUM") as proj_ps,
            tc.tile_pool(name="xin", bufs=xin_bufs) as xin,
            tc.tile_pool(name="xtp", bufs=xtp_bufs) as xtp,
            tc.tile_pool(name="qk", bufs=qk_bufs) as qk,
        ):
            xt_slabs = {}

            def get_slab(sl):
                if sl not in xt_slabs:
                    xt_slabs[sl] = xtp.tile(
                        [P, CC, SLAB], XDT, tag="xT", name=f"xt_{sl}"
                    )
                return xt_slabs[sl]

            def emit_transpose(t):
                """Transpose x tile t via PE, evacuate into its slab."""
                sl, tt = divmod(t, TPS)
                if x_resident:
                    x_t = x_sb[:, t, :]
                else:
                    x_tile = xin.tile([P, C], BF16, tag="x", name=f"x_{t}")
                    x_t = x_tile[:]
                    nc.sync.dma_start(x_t, x[ts(t, P), :])
                pst = tp_ps.tile([P, C], BF16, tag="tp")
                for o in range(CC):
                    nc.tensor.transpose(pst[:, ts(o, P)], x_t[:, ts(o, P)], ident[:])
                nc.vector.tensor_copy(
                    get_slab(sl)[:, :, ts(tt, P)],
                    pst[:].rearrange("p (o n) -> p o n", o=CC),
                )

            emit_transpose(0)
            for t in range(NT):
                sl, tt = divmod(t, TPS)
                xt_slab = get_slab(sl)
                if t + 1 < NT:
                    emit_transpose(t + 1)

                # q = relu(x @ Wq + bq), k = relu(x @ Wk + bk)
                # (paired per chunk so each stationary x^T tile is
                # loaded once for both projections)
                q_ps = proj_ps.tile([P, C], F32, tag="proj")
                k_ps = proj_ps.tile([P, C], F32, tag="proj")
                if proj_fp8:
                    # DoubleRow: contract two 128-channel chunks per matmul.
                    # ilv: emit each projection's bias matmul (bf16, stream-
                    # bound, cheap FWL load) right after its LDW-bound DR
                    # pair so weight loads prefetch under bias streams.
                    def dr_proj(ps, lo, hi):
                        for o2 in range(CC // 2):
                            o = 2 * o2
                            last = False if bias_mm else (o2 == CC // 2 - 1)
                            nc.tensor.matmul(
                                ps[:],
                                xt_slab[:, o : o + 2, ts(tt, P)],
                                wqkv_r[:, o : o + 2, lo:hi],
                                start=(o2 == 0),
                                stop=last,
                                perf_mode=mybir.MatmulPerfMode.DoubleRow,
                            )

                    if ilv and bias_mm:
                        dr_proj(q_ps, 0, C)
                        nc.tensor.matmul(
                            q_ps[:], ones_r[:], brow_r[:, 0:C],
                            start=False, stop=True,
                        )
                        dr_proj(k_ps, C, 2 * C)
                        nc.tensor.matmul(
                            k_ps[:], ones_r[:], brow_r[:, C : 2 * C],
                            start=False, stop=True,
                        )
                    else:
                        for o2 in range(CC // 2):
                            o = 2 * o2
                            last = False if bias_mm else (o2 == CC // 2 - 1)
                            nc.tensor.matmul(
                                q_ps[:],
                                xt_slab[:, o : o + 2, ts(tt, P)],
                                wqkv_r[:, o : o + 2, 0:C],
                                start=(o2 == 0),
                                stop=last,
                                perf_mode=mybir.MatmulPerfMode.DoubleRow,
                            )
                            nc.tensor.matmul(
                                k_ps[:],
                                xt_slab[:, o : o + 2, ts(tt, P)],
                                wqkv_r[:, o : o + 2, C : 2 * C],
                                start=(o2 == 0),
                                stop=last,
                                perf_mode=mybir.MatmulPerfMode.DoubleRow,
                            )
                else:
                    for o in range(CC):
                        nc.tensor.matmul(
                            q_ps[:],
                            xt_slab[:, o, ts(tt, P)],
                            wqkv_r[:, o, 0:C],
                            start=(o == 0),
                            stop=(False if bias_mm else o == CC - 1),
                        )
                        nc.tensor.matmul(
                            k_ps[:],
                            xt_slab[:, o, ts(tt, P)],
                            wqkv_r[:, o, C : 2 * C],
                            start=(o == 0),
                            stop=(False if bias_mm else o == CC - 1),
                        )
                # evacuation destinations: flat [P, C] tiles normally, or
                # slot t%2 of a token-tile-pair [P, 2, C] fp8 tile so the
                # scores matmuls can contract both tiles with DoubleRow
                if full_fp8:
                    if t % 2 == 0:
                        pair_tiles = (
                            qk.tile([P, 2, C], FP8, tag="qpair", name=f"qp_{t}"),
                            qk.tile([P, 2, C], FP8, tag="kpair", name=f"kp_{t}"),
                        )
                    q_pair, k_pair = pair_tiles
                    q_dst = q_pair[:, t % 2, :]
                    k_dst = k_pair[:, t % 2, :]
                else:
                    q_sb = qk.tile([P, C], BF16, tag="qs")
                    k_sb = qk.tile([P, C], BF16, tag="ks")
                    q_dst = q_sb[:]
                    k_dst = k_sb[:]

                dsc = 1.0 / FP8_WSCALE
                if bias_mm:
                    if not (ilv and proj_fp8):
                        nc.tensor.matmul(
                            q_ps[:], ones_r[:], brow_r[:, 0:C],
                            start=False, stop=True,
                        )
                        nc.tensor.matmul(
                            k_ps[:], ones_r[:], brow_r[:, C : 2 * C],
                            start=False, stop=True,
                        )
                    if proj_fp8:
                        nc.scalar.activation(
                            q_dst, q_ps[:], mybir.ActivationFunctionType.Relu,
                            scale=dsc,
                        )
                        nc.vector.tensor_scalar(
                            k_dst, k_ps[:], 0.0, dsc,
                            mybir.AluOpType.max, mybir.AluOpType.mult,
                        )
                    else:
                        nc.scalar.activation(
                            q_dst, q_ps[:], mybir.ActivationFunctionType.Relu
                        )
                        nc.vector.tensor_scalar_max(k_dst, k_ps[:], 0.0)
                else:
                    # (for proj_fp8 the PSUM holds 16x the true pre-bias
                    # values and bq/bk_bc were pre-scaled to match; relu's
                    # positive homogeneity lets us descale after the max)
                    q_tmp = qk.tile([P, C], BF16, tag="qt")
                    nc.vector.tensor_tensor(
                        q_tmp[:], q_ps[:], bq_bc[:], mybir.AluOpType.add
                    )
                    if proj_fp8:
                        nc.scalar.activation(
                            q_dst, q_tmp[:],
                            mybir.ActivationFunctionType.Relu, scale=dsc,
                        )
                    else:
                        nc.scalar.activation(
                            q_dst, q_tmp[:], mybir.ActivationFunctionType.Relu
                        )
                    k_tmp = qk.tile([P, C], BF16, tag="kt")
                    nc.vector.tensor_tensor(
                        k_tmp[:], k_ps[:], bk_bc[:], mybir.AluOpType.add
                    )
                    if proj_fp8:
                        nc.vector.tensor_scalar(
                            k_dst, k_tmp[:], 0.0, dsc,
                            mybir.AluOpType.max, mybir.AluOpType.mult,
                        )
                    else:
                        nc.vector.tensor_scalar_max(k_dst, k_tmp[:], 0.0)

                # scores[o] += q[:, o-chunk]^T @ k
                if full_fp8:
                    if t % 2 == 1:
                        for o in range(CC):
                            nc.tensor.matmul(
                                scores_ps[o][:],
                                q_pair[:, :, ts(o, P)],
                                k_pair[:, :, :],
                                start=(t == 1),
                                stop=(t == NT - 1),
                                perf_mode=mybir.MatmulPerfMode.DoubleRow,
                            )
                else:
                    for o in range(CC):
                        nc.tensor.matmul(
                            scores_ps[o][:],
                            q_sb[:, ts(o, P)],
                            k_sb[:],
                            start=(t == 0),
                            stop=(t == NT - 1),
                        )

                if tt == TPS - 1:
                    # vT[d, n] = relu(Wv^T x^T + bv)  (channel-major)
                    for d in range(CC):
                        v_ps = proj_ps.tile([P, C], F32, tag="proj")
                        if proj_fp8:
                            for o2 in range(CC // 2):
                                o = 2 * o2
                                nc.tensor.matmul(
                                    v_ps[:, :SLAB],
                                    wqkv_r[:, o : o + 2, ds(2 * C + d * P, P)],
                                    xt_slab[:, o : o + 2, :],
                                    start=(o2 == 0),
                                    stop=(o2 == CC // 2 - 1),
                                    perf_mode=mybir.MatmulPerfMode.DoubleRow,
                                )
                            nc.scalar.activation(
                                vt_sb[:, d, ts(sl, SLAB)],
                                v_ps[:, :SLAB],
                                mybir.ActivationFunctionType.Relu,
                                bias=bv[:, d : d + 1],
                                scale=1.0 / FP8_WSCALE,
                            )
                        else:
                            for o in range(CC):
                                nc.tensor.matmul(
                                    v_ps[:, :SLAB],
                                    wqkv_r[:, o, ds(2 * C + d * P, P)],
                                    xt_slab[:, o, :],
                                    start=(o == 0),
                                    stop=(o == CC - 1),
                                )
                            nc.scalar.activation(
                                vt_sb[:, d, ts(sl, SLAB)],
                                v_ps[:, :SLAB],
                                mybir.ActivationFunctionType.Relu,
                                bias=bv[:, d : d + 1],
                            )
                    del xt_slabs[sl]

            # ---- softmax + adjacency gate ------------------------------
            with tc.tile_pool(name="smx", bufs=8) as smx:
                for o in range(CC):
                    smax = smx.tile([P, 1], F32, tag="smax")
                    nc.vector.reduce_max(
                        smax[:], scores_ps[o][:], axis=mybir.AxisListType.X
                    )
                    nbias = smx.tile([P, 1], F32, tag="nbias")
                    nc.vector.tensor_scalar_mul(nbias[:], smax[:], -s)
                    ssum = smx.tile([P, 1], F32, tag="ssum")
                    attn_e = smx.tile([P, C], F32, tag="attn_e")
                    nc.scalar.activation(
                        attn_e[:],
                        scores_ps[o][:],
                        mybir.ActivationFunctionType.Exp,
                        bias=nbias[:],
                        scale=s,
                        accum_out=ssum[:],
                    )
                    rsum = smx.tile([P, 1], F32, tag="rsum")
                    nc.vector.reciprocal(rsum[:], ssum[:])
                    attn_r = smx.tile([P, C], F32, tag="attn_r")
                    nc.vector.tensor_scalar_mul(attn_r[:], attn_e[:], rsum[:])
                    nc.vector.tensor_mul(
                        attn_sb[:, o, :], attn_r[:], adj_sb[:, o, :]
                    )

        # ---- pass 2: y = v @ attn ; out = y @ Wo + bo ------------------
        with (
            tc.tile_pool(name="y_ps", bufs=y_bufs, space="PSUM") as y_ps_pool,
            tc.tile_pool(name="yt", bufs=2) as ytp,
            tc.tile_pool(name="outp", bufs=3) as outp,
        ):
            def emit_yt(sl):
                yt_slab = ytp.tile([P, CC, SLAB], BF16, tag="yT", name=f"yt_{sl}")
                for d in range(CC):
                    y_ps = y_ps_pool.tile([P, C], F32, tag="y", name=f"y_{sl}_{d}")
                    if full_fp8:
                        for o2 in range(CC // 2):
                            o = 2 * o2
                            nc.tensor.matmul(
                                y_ps[:, :SLAB],
                                attn_sb[:, o : o + 2, ts(d, P)],
                                vt_sb[:, o : o + 2, ts(sl, SLAB)],
                                start=(o2 == 0),
                                stop=(o2 == CC // 2 - 1),
                                perf_mode=mybir.MatmulPerfMode.DoubleRow,
                            )
                    else:
                        for o in range(CC):
                            nc.tensor.matmul(
                                y_ps[:, :SLAB],
                                attn_sb[:, o, ts(d, P)],
                                vt_sb[:, o, ts(sl, SLAB)],
                                start=(o == 0),
                                stop=(o == CC - 1),
                            )
                    nc.scalar.copy(yt_slab[:, d, :], y_ps[:, :SLAB])
                return yt_slab

            def emit_out(sl, yt_slab):
                for tt in range(TPS):
                    t = sl * TPS + tt
                    o_ps = y_ps_pool.tile([P, C], F32, tag="y", name=f"o_{sl}_{tt}")
                    for d in range(CC):
                        nc.tensor.matmul(
                            o_ps[:],
                            yt_slab[:, d, ts(tt, P)],
                            wo_r[:, d, :],
                            start=(d == 0),
                            stop=(d == CC - 1),
                        )
                    out_sb = outp.tile([P, C], F32, tag="out", name=f"os_{sl}_{tt}")
                    nc.vector.tensor_tensor(
                        out_sb[:], o_ps[:], bo_bc[:], mybir.AluOpType.add
                    )
                    nc.sync.dma_start(out[ts(t, P), :], out_sb[:])

            def emit_yt_d(sl, yt_slab, d):
                y_ps = y_ps_pool.tile([P, C], F32, tag="y", name=f"y_{sl}_{d}")
                if full_fp8:
                    for o2 in range(CC // 2):
                        o = 2 * o2
                        nc.tensor.matmul(
                            y_ps[:, :SLAB],
                            attn_sb[:, o : o + 2, ts(d, P)],
                            vt_sb[:, o : o + 2, ts(sl, SLAB)],
                            start=(o2 == 0),
                            stop=(o2 == CC // 2 - 1),
                            perf_mode=mybir.MatmulPerfMode.DoubleRow,
                        )
                else:
                    for o in range(CC):
                        nc.tensor.matmul(
                            y_ps[:, :SLAB],
                            attn_sb[:, o, ts(d, P)],
                            vt_sb[:, o, ts(sl, SLAB)],
                            start=(o == 0),
                            stop=(o == CC - 1),
                        )
                nc.scalar.copy(yt_slab[:, d, :], y_ps[:, :SLAB])

            def emit_out_tt(sl, yt_slab, tt):
                t = sl * TPS + tt
                o_ps = y_ps_pool.tile([P, C], F32, tag="y", name=f"o_{sl}_{tt}")
                for d in range(CC):
                    nc.tensor.matmul(
                        o_ps[:],
                        yt_slab[:, d, ts(tt, P)],
                        wo_r[:, d, :],
                        start=(d == 0),
                        stop=(d == CC - 1),
                    )
                out_sb = outp.tile([P, C], F32, tag="out", name=f"os_{sl}_{tt}")
                nc.vector.tensor_tensor(
                    out_sb[:], o_ps[:], bo_bc[:], mybir.AluOpType.add
                )
                nc.sync.dma_start(out[ts(t, P), :], out_sb[:])

            if ilv:
                # round-robin the LDW-bound yt DR blocks with the stream-
                # bound out-stage bf16 blocks of the previous slab
                prev = None
                for sl in range(NS):
                    yt_slab = ytp.tile(
                        [P, CC, SLAB], BF16, tag="yT", name=f"yt_{sl}"
                    )
                    for i in range(CC):
                        emit_yt_d(sl, yt_slab, i)
                        if prev is not None:
                            emit_out_tt(sl - 1, prev, i)
                    prev = yt_slab
                for i in range(TPS):
                    emit_out_tt(NS - 1, prev, i)
            elif pipe_p2:
                prev = None
                for sl in range(NS):
                    yt_slab = emit_yt(sl)
                    if prev is not None:
                        emit_out(sl - 1, prev)
                    prev = yt_slab
                emit_out(NS - 1, prev)
            else:
                for sl in range(NS):
                    emit_out(sl, emit_yt(sl))

    nc.compile()
    return nc


def _get_nc(reps: int = 1, **kw):
    eff = {**DEFAULTS, **kw}
    key = ("nc", reps, tuple(sorted(eff.items())))
    if key not in _CACHE:
        _CACHE[key] = build(reps, **eff)
    return _CACHE[key]


def _run(inputs, trace=False, reps: int = 1, **kw):
    eff = {**DEFAULTS, **kw}
    nc = _get_nc(reps, **kw)
    xf = np.ascontiguousarray(np.asarray(inputs["x"], dtype=np.float32))
    wqkvf = np.ascontiguousarray(np.asarray(inputs["Wqkv"], dtype=np.float32))
    x = xf.astype(NP_BF16)
    if eff.get("proj_fp8") or eff.get("full_fp8"):
        wqkv = (wqkvf * FP8_WSCALE).astype(NP_FP8)
    else:
        wqkv = wqkvf.astype(NP_BF16)
    adj = np.ascontiguousarray(np.asarray(inputs["adj"], dtype=np.float32))
    bqkv = np.ascontiguousarray(np.asarray(inputs["bqkv"], dtype=np.float32))
    wof = np.ascontiguousarray(np.asarray(inputs["Wo"], dtype=np.float32))
    if eff.get("full_fp8"):
        # attn carries x256 (folded into adj); undo via Wo/256
        adj = adj * ATTN_SCALE
        wo = (wof / ATTN_SCALE).astype(NP_BF16)
    else:
        wo = wof.astype(NP_BF16)
    bo = np.ascontiguousarray(np.asarray(inputs["bo"], dtype=np.float32))

    in_maps = [
        {
            "x": x[b],
            "adj": adj[b],
            "Wqkv": wqkv,
            "bqkv": bqkv,
            "Wo": wo,
            "bo": bo,
        }
        for b in range(B)
    ]
    res = run_bass_kernel_spmd(
        nc, in_maps, core_ids=list(range(B)), trace=trace
    )
    outp = np.stack([res.results[b]["out"] for b in range(B)], axis=0)
    return outp.astype(np.float32), res


def kernel(**inputs) -> np.ndarray:
    out, _ = _run(inputs, trace=False)
    return out
